# Initial kernel scaffold
#
"""Trainium2 Bass kernel for the double-FIR "DeconvLayer" problem.

Reference computation (see problem statement):
    v = mask(conv(x, k1)),  y = mask(conv(v, k2))
with k1 = [1, h], k2 = [1, h_reversed], mask zeroing columns < 16.

Key facts used here:
  * For output column t >= 48 the two masked passes are EXACTLY one causal
    33-tap conv with kc = full_conv(k1, k2):  y[t] = sum_d kc[d] x[t-d].
  * y[:, 0:128] depends only on x[:, 0:128] and is a fixed linear map M0
    (built numerically from the reference semantics, including the masks).

Device strategy (pure data parallel, 32 batch rows / core, 8 cores):
  * Host marshals x into time-major columns: each column of the device
    input is one aligned 128-sample block of one row.  Then
        y_block(c) = A.T @ x_block(c) + C.T @ x_block(c-1)
    where A[p, po] = kc[po - p] (banded, in-block taps) and
    C[p, po] = kc[po + 128 - p] (top-right corner, halo taps from the
    previous block).  A and C are STATIONARY operands; the data streams
    as the moving operand in N=512 column tiles -> no transposes on chip,
    perfectly contiguous DMA both directions.
  * Row-initial blocks get garbage halo from the preceding row; their
    first 128 outputs are instead produced exactly by an extra matmul
    with the dense M0 matrix on separately-staged copies of each row's
    block 0, and the host takes those columns for y[:, 0:128].
  * fp32 matmul runs at 1/4 rate on TRN2, so x and the band matrices are
    split into fp16 hi+lo parts; 3 of the 4 cross terms are kept
    (hi*hi + hi*lo + lo*hi), giving ~1e-6 relative error at full fp16
    matmul rate.  DMA bytes are identical to fp32 (2 x fp16 arrays).
  * Engines used: DMA (in/out), PE (matmuls), DVE (PSUM->SBUF copies).
    Per core: ~16.8 MB in + ~16.8 MB out => memory-bound as targeted.

The harness calls kernel(**inputs) with the FULL inputs; everything the
device needs (shapes, tiling) is hardcoded below.
"""

import numpy as np

import concourse.bass as bass  # noqa: F401  (dtype/AP helpers)
import concourse.mybir as mybir
from concourse import bacc
from concourse.bass_utils import run_bass_kernel_spmd
from concourse.tile import TileContext

# ---------------------------------------------------------------- geometry
B, N, F = 256, 131072, 16
N_CORES = 8
RPC = B // N_CORES          # 32 batch rows per core
BLK = 128                   # time block = matmul contraction window
BPR = N // BLK              # 1024 blocks per row
DCOLS = RPC * BPR           # 32768 data columns per core
MT = 2048                   # macro-tile: columns per in/out DMA
NMT = DCOLS // MT           # 16 macro tiles per core
PT = 512                    # psum tile columns (one fp32 PSUM bank)
GRP = 4                     # psum tiles per weight-reuse group
W_IN = MT + 1               # input window cols per macro tile (+1 for corner)
PACK = 2 * W_IN             # hi + lo packed per macro tile
XCOLS = NMT * PACK + 2 * RPC  # packed device input width
YCOLS = 1 + DCOLS + RPC     # device output: [0] dead, main, first-block tail

_F16 = mybir.dt.float16
_F32 = mybir.dt.float32

# ------------------------------------------------------------- bass program
_NC_CACHE = None


def _build_nc(repeats=1):
    nc = bacc.Bacc()
    xp = nc.dram_tensor("xp", [128, XCOLS], _F16, kind="ExternalInput")
    wm = nc.dram_tensor("wm", [128, 768], _F16, kind="ExternalInput")
    yT = nc.dram_tensor("yT", [128, YCOLS], _F32, kind="ExternalOutput")

    with TileContext(nc) as tc:
        with (
            tc.tile_pool(name="w", bufs=1) as wpool,
            tc.tile_pool(name="xin", bufs=4) as xpool,
            tc.tile_pool(name="stage", bufs=4) as spool,
            tc.tile_pool(name="ps", bufs=8, space="PSUM") as pspool,
        ):
            wsb = wpool.tile([128, 768], _F16, tag="wsb")
            nc.sync.dma_start(out=wsb[:], in_=wm[:, :])
            A_hi = wsb[:, 0:128]
            A_lo = wsb[:, 128:256]
            C_hi = wsb[:, 256:384]
            C_lo = wsb[:, 384:512]
            M_hi = wsb[:, 512:640]
            M_lo = wsb[:, 640:768]

            for m in range(NMT * repeats):
                m = m % NMT
                xt = xpool.tile([128, PACK], _F16, tag="xt")
                nc.sync.dma_start(
                    out=xt[:], in_=xp[:, m * PACK : (m + 1) * PACK]
                )
                th = xt[:, 0:W_IN]
                tl = xt[:, W_IN:PACK]
                stage = spool.tile([128, MT], _F32, tag="stage")

                for g in range(MT // (PT * GRP)):
                    pss = [
                        pspool.tile(
                            [128, PT], _F32, name=f"ps_{m}_{g}_{j}", tag="ps"
                        )
                        for j in range(GRP)
                    ]

                    def _sl(j, _g=g):
                        t0 = (_g * GRP + j) * PT
                        return slice(t0 + 1, t0 + 1 + PT), slice(t0, t0 + PT)

                    # Weight-grouped passes: 4 LDWEIGHTS per 24 matmuls.
                    for j, ps in enumerate(pss):
                        cur, _ = _sl(j)
                        nc.tensor.matmul(
                            ps[:], A_hi, th[:, cur], start=True, stop=False
                        )
                    for j, ps in enumerate(pss):
                        cur, _ = _sl(j)
                        nc.tensor.matmul(
                            ps[:], A_hi, tl[:, cur], start=False, stop=False
                        )
                    for j, ps in enumerate(pss):
                        cur, _ = _sl(j)
                        nc.tensor.matmul(
                            ps[:], A_lo, th[:, cur], start=False, stop=False
                        )
                    for j, ps in enumerate(pss):
                        _, prv = _sl(j)
                        nc.tensor.matmul(
                            ps[:], C_hi, th[:, prv], start=False, stop=False
                        )
                    for j, ps in enumerate(pss):
                        _, prv = _sl(j)
                        nc.tensor.matmul(
                            ps[:], C_hi, tl[:, prv], start=False, stop=False
                        )
                    for j, ps in enumerate(pss):
                        _, prv = _sl(j)
                        nc.tensor.matmul(
                            ps[:], C_lo, th[:, prv], start=False, stop=True
                        )
                    for j, ps in enumerate(pss):
                        t0 = (g * GRP + j) * PT
                        if j % 2 == 0:
                            nc.vector.tensor_copy(
                                out=stage[:, t0 : t0 + PT], in_=ps[:]
                            )
                        else:
                            nc.scalar.copy(
                                out=stage[:, t0 : t0 + PT], in_=ps[:]
                            )

                nc.sync.dma_start(
                    out=yT[:, 1 + m * MT : 1 + (m + 1) * MT], in_=stage[:]
                )

            # Exact first 128 samples of every row via the dense M0 map.
            et = xpool.tile([128, 2 * RPC], _F16, tag="extra")
            nc.sync.dma_start(out=et[:], in_=xp[:, NMT * PACK : XCOLS])
            eh = et[:, 0:RPC]
            el = et[:, RPC : 2 * RPC]
            ps2 = pspool.tile([128, RPC], _F32, name="ps2", tag="ps")
            nc.tensor.matmul(ps2[:], M_hi, eh, start=True, stop=False)
            nc.tensor.matmul(ps2[:], M_lo, eh, start=False, stop=False)
            nc.tensor.matmul(ps2[:], M_hi, el, start=False, stop=True)
            st2 = spool.tile([128, RPC], _F32, tag="st2")
            nc.vector.tensor_copy(out=st2[:], in_=ps2[:])
            nc.sync.dma_start(out=yT[:, 1 + DCOLS : YCOLS], in_=st2[:])

    nc.compile()  # bacc legalization: ≤1 sync wait per HW instruction
    return nc


def _get_nc():
    global _NC_CACHE
    if _NC_CACHE is None:
        _NC_CACHE = _build_nc()
    return _NC_CACHE


# ------------------------------------------------------------- host helpers
def _hi_lo(a64):
    hi = a64.astype(np.float16)
    lo = (a64 - hi.astype(np.float64)).astype(np.float16)
    return hi, lo


def _fir_mat(taps):
    """128x128 matrix of one masked FIR pass: y = T @ x (first 128 samples).

    y[i] = x[i] + sum_j taps[j] * x[i-j-1] for i >= F, else 0.
    """
    T = np.zeros((128, 128))
    for i in range(F, 128):
        T[i, i] = 1.0
        for j in range(F):
            T[i, i - j - 1] += taps[j]
    return T


def _build_weights(h64):
    """Stationary operands, stacked as [A_hi A_lo C_hi C_lo M_hi M_lo]."""
    k1 = np.concatenate([[1.0], h64])
    k2 = np.concatenate([[1.0], h64[::-1]])
    kc = np.convolve(k1, k2)  # 33 taps

    i = np.arange(128)
    D = i[None, :] - i[:, None]  # po - p
    A = np.zeros((128, 128))
    mask = (D >= 0) & (D <= 32)
    A[mask] = kc[D[mask]]
    Dc = D + 128
    C = np.zeros((128, 128))
    maskc = (Dc >= 0) & (Dc <= 32)
    C[maskc] = kc[Dc[maskc]]

    M0 = (_fir_mat(h64[::-1]) @ _fir_mat(h64)).T  # M0[p, po] = dy[po]/dx[p]

    wm = np.zeros((128, 768), np.float16)
    for k, mat in enumerate([A, C, M0]):
        hi, lo = _hi_lo(mat)
        wm[:, k * 256 : k * 256 + 128] = hi
        wm[:, k * 256 + 128 : k * 256 + 256] = lo
    return wm


def _make_in_maps(x, h64):
    wm = _build_weights(h64)
    in_maps = []
    for c in range(N_CORES):
        xs = np.ascontiguousarray(x[c * RPC : (c + 1) * RPC])  # (32, 131072)
        Bv = xs.reshape(RPC, BPR, BLK)
        xin = np.zeros((128, 1 + DCOLS), np.float32)
        xin[:, 1:] = Bv.transpose(2, 0, 1).reshape(BLK, DCOLS)
        hi = xin.astype(np.float16)
        lo = (xin - hi.astype(np.float32)).astype(np.float16)
        xpk = np.empty((128, XCOLS), np.float16)
        for m in range(NMT):
            w0 = m * MT
            xpk[:, m * PACK : m * PACK + W_IN] = hi[:, w0 : w0 + W_IN]
            xpk[:, m * PACK + W_IN : (m + 1) * PACK] = lo[:, w0 : w0 + W_IN]
        blk0 = Bv[:, 0, :].astype(np.float32)  # (32, 128)
        b_hi = blk0.astype(np.float16)
        b_lo = (blk0 - b_hi.astype(np.float32)).astype(np.float16)
        xpk[:, NMT * PACK : NMT * PACK + RPC] = b_hi.T
        xpk[:, NMT * PACK + RPC : XCOLS] = b_lo.T
        in_maps.append({"xp": xpk, "wm": wm})
    return in_maps


def _assemble(results):
    y = np.empty((B, N), np.float32)
    for c in range(N_CORES):
        yT = results[c]["yT"]
        main = (
            yT[:, 1 : 1 + DCOLS]
            .reshape(BLK, RPC, BPR)
            .transpose(1, 2, 0)
            .reshape(RPC, N)
        )
        y[c * RPC : (c + 1) * RPC] = main
        y[c * RPC : (c + 1) * RPC, 0:BLK] = yT[:, 1 + DCOLS : YCOLS].T
    return y


def _run(x, h, **spmd_kwargs):
    x = np.asarray(x, dtype=np.float32)
    h64 = np.asarray(h, dtype=np.float64).reshape(-1)
    assert x.shape == (B, N) and h64.shape == (F,)
    in_maps = _make_in_maps(x, h64)
    res = run_bass_kernel_spmd(
        _get_nc(), in_maps, core_ids=list(range(N_CORES)), **spmd_kwargs
    )
    return _assemble(res.results), res


def kernel(x, kernel):
    y, _ = _run(x, kernel)
    return y



# revision 8
# speedup vs baseline: 1.7409x; 1.7409x over previous
"""Trainium2 Bass kernel for the double-FIR "DeconvLayer" problem.

Reference computation (see problem statement):
    v = mask(conv(x, k1)),  y = mask(conv(v, k2))
with k1 = [1, h], k2 = [1, h_reversed], mask zeroing columns < 16.

Key facts used here:
  * For output column t >= 48 the two masked passes are EXACTLY one causal
    33-tap conv with kc = full_conv(k1, k2):  y[t] = sum_d kc[d] x[t-d].
  * y[:, 0:128] depends only on x[:, 0:128] and is a fixed linear map M0
    (built numerically from the reference semantics, including the masks).

Device strategy (pure data parallel, 32 batch rows / core, 8 cores):
  * Host marshals x into time-major columns: each column of the device
    input is one aligned 128-sample block of one row.  Then
        y_block(c) = A.T @ x_block(c) + C.T @ x_block(c-1)
    where A[p, po] = kc[po - p] (banded, in-block taps) and
    C[p, po] = kc[po + 128 - p] (top-right corner, halo taps from the
    previous block).  A and C are STATIONARY operands; the data streams
    as the moving operand in 512-column PSUM tiles -> no transposes on
    chip, perfectly contiguous DMA both directions.
  * Row-initial blocks get garbage halo from the preceding row; their
    first 128 outputs are instead produced exactly by an extra matmul
    with the dense M0 matrix on separately-staged copies of each row's
    block 0, and the host takes those columns for y[:, 0:128].
  * Precision: fp16 end-to-end on the wire (x cast to fp16 on host, y
    returned as fp16 and upcast on host), fp32 PSUM accumulation.
    Relative L2 error ~4e-4 vs the fp32 reference - well inside the
    2e-2 gate.  This halves both DMA streams vs fp32 I/O and needs only
    2 matmul passes (A, C) instead of 6 hi/lo passes -> both the DMA
    floor (~17 MB/core) and the PE time (~27 us) drop by 2-3x.
  * Engines used: DMA (in/out), PE (matmuls), DVE+ACT (PSUM->SBUF casts).

The harness calls kernel(**inputs) with the FULL inputs; everything the
device needs (shapes, tiling) is hardcoded below.
"""

import numpy as np

import concourse.bass as bass  # noqa: F401  (dtype/AP helpers)
import concourse.mybir as mybir
from concourse import bacc
from concourse.bass_utils import run_bass_kernel_spmd
from concourse.tile import TileContext

# ---------------------------------------------------------------- geometry
B, N, F = 256, 131072, 16
N_CORES = 8
RPC = B // N_CORES          # 32 batch rows per core
BLK = 128                   # time block = matmul contraction window
BPR = N // BLK              # 1024 blocks per row
DCOLS = RPC * BPR           # 32768 data columns per core
MT = 8192                   # macro-tile: columns per in/out DMA
NMT = DCOLS // MT           # 4 macro tiles per core
PT = 512                    # psum tile columns (one fp32 PSUM bank)
GRP = 4                     # psum tiles per weight-reuse group
W_IN = MT + 16              # input cols per macro tile (halo + align pad)
XCOLS = NMT * W_IN + RPC    # device input width (+32 row-initial blocks)
YPAD = 16                   # dead lead-in keeps out-DMA rows 32B-aligned
YCOLS = YPAD + DCOLS + RPC  # device output: pad, main, first-block tail

_F16 = mybir.dt.float16
_F32 = mybir.dt.float32

# ------------------------------------------------------------- bass program
_NC_CACHE = None


def _build_nc():
    nc = bacc.Bacc()
    xp = nc.dram_tensor("xp", [128, XCOLS], _F16, kind="ExternalInput")
    wm = nc.dram_tensor("wm", [128, 384], _F16, kind="ExternalInput")
    yT = nc.dram_tensor("yT", [128, YCOLS], _F16, kind="ExternalOutput")

    with TileContext(nc) as tc:
        with (
            tc.tile_pool(name="w", bufs=1) as wpool,
            tc.tile_pool(name="xin", bufs=4) as xpool,
            tc.tile_pool(name="stage", bufs=3) as spool,
            tc.tile_pool(name="ps", bufs=8, space="PSUM") as pspool,
        ):
            wsb = wpool.tile([128, 384], _F16, tag="wsb")
            nc.sync.dma_start(out=wsb[:], in_=wm[:, :])
            A = wsb[:, 0:128]
            C = wsb[:, 128:256]
            M0 = wsb[:, 256:384]

            # Exact first 128 samples of every row via the dense M0 map.
            # Scheduled first so the epilogue is just the last out-DMA.
            et = xpool.tile([128, RPC], _F16, tag="extra")
            nc.sync.dma_start(out=et[:], in_=xp[:, NMT * W_IN : XCOLS])
            ps2 = pspool.tile([128, RPC], _F32, name="ps2", tag="ps")
            nc.tensor.matmul(ps2[:], M0, et[:], start=True, stop=True)
            st2 = spool.tile([128, RPC], _F16, tag="st2")
            nc.vector.tensor_copy(out=st2[:], in_=ps2[:])
            # Output DMAs ride the Scalar engine's HWDGE queue so the input
            # prefetch stream on Sync is never blocked behind their waits.
            nc.scalar.dma_start(out=yT[:, YPAD + DCOLS : YCOLS], in_=st2[:])

            for m in range(NMT):
                xt = xpool.tile([128, W_IN], _F16, tag="xt")
                nc.sync.dma_start(
                    out=xt[:], in_=xp[:, m * W_IN : (m + 1) * W_IN]
                )
                stage = spool.tile([128, MT], _F16, tag="stage")

                for g in range(MT // (PT * GRP)):
                    pss = [
                        pspool.tile(
                            [128, PT], _F32, name=f"ps_{m}_{g}_{j}", tag="ps"
                        )
                        for j in range(GRP)
                    ]

                    # Weight-grouped passes: 2 weight loads per 8 matmuls.
                    for j, ps in enumerate(pss):
                        t0 = (g * GRP + j) * PT
                        nc.tensor.matmul(
                            ps[:], A, xt[:, 16 + t0 : 16 + t0 + PT],
                            start=True, stop=False,
                        )
                    for j, ps in enumerate(pss):
                        t0 = (g * GRP + j) * PT
                        nc.tensor.matmul(
                            ps[:], C, xt[:, 15 + t0 : 15 + t0 + PT],
                            start=False, stop=True,
                        )
                    for j, ps in enumerate(pss):
                        t0 = (g * GRP + j) * PT
                        if j % 2 == 0:
                            nc.vector.tensor_copy(
                                out=stage[:, t0 : t0 + PT], in_=ps[:]
                            )
                        else:
                            nc.scalar.copy(
                                out=stage[:, t0 : t0 + PT], in_=ps[:]
                            )

                nc.scalar.dma_start(
                    out=yT[:, YPAD + m * MT : YPAD + (m + 1) * MT],
                    in_=stage[:],
                )

    nc.compile()  # bacc legalization: <=1 sync wait per HW instruction
    return nc


def _get_nc():
    global _NC_CACHE
    if _NC_CACHE is None:
        _NC_CACHE = _build_nc()
    return _NC_CACHE


# ------------------------------------------------------------- host helpers
def _fir_mat(taps):
    """128x128 matrix of one masked FIR pass: y = T @ x (first 128 samples).

    y[i] = x[i] + sum_j taps[j] * x[i-j-1] for i >= F, else 0.
    """
    T = np.zeros((128, 128))
    for i in range(F, 128):
        T[i, i] = 1.0
        for j in range(F):
            T[i, i - j - 1] += taps[j]
    return T


def _build_weights(h64):
    """Stationary operands, stacked as [A C M0], fp16."""
    k1 = np.concatenate([[1.0], h64])
    k2 = np.concatenate([[1.0], h64[::-1]])
    kc = np.convolve(k1, k2)  # 33 taps

    i = np.arange(128)
    D = i[None, :] - i[:, None]  # po - p
    A = np.zeros((128, 128))
    mask = (D >= 0) & (D <= 32)
    A[mask] = kc[D[mask]]
    Dc = D + 128
    C = np.zeros((128, 128))
    maskc = (Dc >= 0) & (Dc <= 32)
    C[maskc] = kc[Dc[maskc]]

    M0 = (_fir_mat(h64[::-1]) @ _fir_mat(h64)).T  # M0[p, po] = dy[po]/dx[p]

    wm = np.zeros((128, 384), np.float16)
    wm[:, 0:128] = A
    wm[:, 128:256] = C
    wm[:, 256:384] = M0
    return wm


def _make_in_maps(x, h64):
    wm = _build_weights(h64)
    in_maps = []
    for c in range(N_CORES):
        xs = np.ascontiguousarray(x[c * RPC : (c + 1) * RPC])  # (32, 131072)
        Bv = xs.reshape(RPC, BPR, BLK)
        xin = np.zeros((128, 1 + DCOLS), np.float16)
        xin[:, 1:] = Bv.transpose(2, 0, 1).reshape(BLK, DCOLS)
        xpk = np.zeros((128, XCOLS), np.float16)
        for m in range(NMT):
            # halo col at slot 15, current cols at slots 16..16+MT-1
            xpk[:, m * W_IN + 15 : (m + 1) * W_IN] = (
                xin[:, m * MT : m * MT + MT + 1]
            )
        xpk[:, NMT * W_IN : XCOLS] = Bv[:, 0, :].astype(np.float16).T
        in_maps.append({"xp": xpk, "wm": wm})
    return in_maps


def _assemble(results):
    y = np.empty((B, N), np.float32)
    for c in range(N_CORES):
        yT = results[c]["yT"].astype(np.float32)
        main = (
            yT[:, YPAD : YPAD + DCOLS]
            .reshape(BLK, RPC, BPR)
            .transpose(1, 2, 0)
            .reshape(RPC, N)
        )
        y[c * RPC : (c + 1) * RPC] = main
        y[c * RPC : (c + 1) * RPC, 0:BLK] = yT[:, YPAD + DCOLS : YCOLS].T
    return y


def _run(x, h, **spmd_kwargs):
    x = np.asarray(x, dtype=np.float32)
    h64 = np.asarray(h, dtype=np.float64).reshape(-1)
    assert x.shape == (B, N) and h64.shape == (F,)
    in_maps = _make_in_maps(x, h64)
    res = run_bass_kernel_spmd(
        _get_nc(), in_maps, core_ids=list(range(N_CORES)), **spmd_kwargs
    )
    return _assemble(res.results), res


def kernel(x, kernel):
    y, _ = _run(x, kernel)
    return y


# revision 10
# speedup vs baseline: 2.0202x; 1.1604x over previous
"""Trainium2 Bass kernel for the double-FIR "DeconvLayer" problem.

Reference computation (see problem statement):
    v = mask(conv(x, k1)),  y = mask(conv(v, k2))
with k1 = [1, h], k2 = [1, h_reversed], mask zeroing columns < 16.

Key facts used here:
  * For output column t >= 48 the two masked passes are EXACTLY one causal
    33-tap conv with kc = full_conv(k1, k2):  y[t] = sum_d kc[d] x[t-d].
  * y[:, 0:128] depends only on x[:, 0:128] and is a fixed linear map M0
    (built numerically from the reference semantics, including the masks).

Device strategy (pure data parallel, 32 batch rows / core, 8 cores):
  * Host marshals x into time-major columns: each column of the device
    input is one aligned 128-sample block of one row.  Then
        y_block(c) = A.T @ x_block(c) + C.T @ x_block(c-1)
    where A[p, po] = kc[po - p] (banded, in-block taps) and
    C[p, po] = kc[po + 128 - p] (top-right corner, halo taps from the
    previous block).  A and C are STATIONARY operands; the data streams
    as the moving operand in 512-column PSUM tiles -> no transposes on
    chip, perfectly contiguous DMA both directions.
  * Row-initial blocks get garbage halo from the preceding row; their
    first 128 outputs are instead produced exactly by an extra matmul
    with the dense M0 matrix on separately-staged copies of each row's
    block 0, and the host takes those columns for y[:, 0:128].
  * Precision: fp16 end-to-end on the wire (x cast to fp16 on host, y
    returned as fp16 and upcast on host), fp32 PSUM accumulation.
    Relative L2 error ~4e-4 vs the fp32 reference - well inside the
    2e-2 gate.  This halves both DMA streams vs fp32 I/O and needs only
    2 matmul passes (A, C) instead of 6 hi/lo passes -> both the DMA
    floor (~17 MB/core) and the PE time (~27 us) drop by 2-3x.
  * Engines used: DMA (in/out), PE (matmuls), DVE+ACT (PSUM->SBUF casts).

The harness calls kernel(**inputs) with the FULL inputs; everything the
device needs (shapes, tiling) is hardcoded below.
"""

import numpy as np

import concourse.bass as bass  # noqa: F401  (dtype/AP helpers)
import concourse.mybir as mybir
from concourse import bacc
from concourse.bass_utils import run_bass_kernel_spmd
from concourse.tile import TileContext

# ---------------------------------------------------------------- geometry
B, N, F = 256, 131072, 16
N_CORES = 8
RPC = B // N_CORES          # 32 batch rows per core
BLK = 128                   # time block = matmul contraction window
BPR = N // BLK              # 1024 blocks per row
DCOLS = RPC * BPR           # 32768 data columns per core
MT = 4096                   # macro-tile: columns per in/out DMA
NMT = DCOLS // MT           # 8 macro tiles per core
PT = 512                    # psum tile columns (one fp32 PSUM bank)
GRP = 4                     # psum tiles per weight-reuse group
W_IN = MT + 16              # input cols per macro tile (halo + align pad)
XCOLS = NMT * W_IN + RPC    # device input width (+32 row-initial blocks)
YPAD = 16                   # dead lead-in keeps out-DMA rows 32B-aligned
YCOLS = YPAD + DCOLS + RPC  # device output: pad, main, first-block tail

_F16 = mybir.dt.float16
_F32 = mybir.dt.float32

# ------------------------------------------------------------- bass program
_NC_CACHE = None


def _build_nc():
    nc = bacc.Bacc()
    xp = nc.dram_tensor("xp", [128, XCOLS], _F16, kind="ExternalInput")
    wm = nc.dram_tensor("wm", [128, 384], _F16, kind="ExternalInput")
    yT = nc.dram_tensor("yT", [128, YCOLS], _F16, kind="ExternalOutput")

    with TileContext(nc) as tc:
        with (
            tc.tile_pool(name="w", bufs=1) as wpool,
            tc.tile_pool(name="xin", bufs=4) as xpool,
            tc.tile_pool(name="stage", bufs=3) as spool,
            tc.tile_pool(name="ps", bufs=8, space="PSUM") as pspool,
        ):
            # First input chunk goes out before anything else on the queue
            # so the pipeline fill starts at t=0 of the DMA stream.
            HC = W_IN // 2 + 8  # chunk A cols (covers group 0 + halo)
            xt0 = xpool.tile([128, W_IN], _F16, tag="xt")
            nc.sync.dma_start(out=xt0[:, 0:HC], in_=xp[:, 0:HC])

            wsb = wpool.tile([128, 384], _F16, tag="wsb")
            nc.sync.dma_start(out=wsb[:], in_=wm[:, :])
            A = wsb[:, 0:128]
            C = wsb[:, 128:256]
            M0 = wsb[:, 256:384]

            # Exact first 128 samples of every row via the dense M0 map.
            # Scheduled first so the epilogue is just the last out-DMA.
            et = xpool.tile([128, RPC], _F16, tag="extra")
            nc.sync.dma_start(out=et[:], in_=xp[:, NMT * W_IN : XCOLS])
            ps2 = pspool.tile([128, RPC], _F32, name="ps2", tag="ps")
            nc.tensor.matmul(ps2[:], M0, et[:], start=True, stop=True)
            st2 = spool.tile([128, RPC], _F16, tag="st2")
            nc.vector.tensor_copy(out=st2[:], in_=ps2[:])
            # Output DMAs ride the Scalar engine's HWDGE queue so the input
            # prefetch stream on Sync is never blocked behind their waits.
            nc.scalar.dma_start(out=yT[:, YPAD + DCOLS : YCOLS], in_=st2[:])

            for m in range(NMT):
                if m == 0:
                    xt = xt0
                    nc.sync.dma_start(
                        out=xt[:, HC:W_IN], in_=xp[:, HC:W_IN]
                    )
                else:
                    xt = xpool.tile([128, W_IN], _F16, tag="xt")
                    w0 = m * W_IN
                    nc.sync.dma_start(
                        out=xt[:, 0:HC], in_=xp[:, w0 : w0 + HC]
                    )
                    nc.sync.dma_start(
                        out=xt[:, HC:W_IN], in_=xp[:, w0 + HC : w0 + W_IN]
                    )
                stage = spool.tile([128, MT], _F16, tag="stage")

                for g in range(MT // (PT * GRP)):
                    pss = [
                        pspool.tile(
                            [128, PT], _F32, name=f"ps_{m}_{g}_{j}", tag="ps"
                        )
                        for j in range(GRP)
                    ]

                    # Weight-grouped passes: 2 weight loads per 8 matmuls.
                    for j, ps in enumerate(pss):
                        t0 = (g * GRP + j) * PT
                        nc.tensor.matmul(
                            ps[:], A, xt[:, 16 + t0 : 16 + t0 + PT],
                            start=True, stop=False,
                        )
                    for j, ps in enumerate(pss):
                        t0 = (g * GRP + j) * PT
                        nc.tensor.matmul(
                            ps[:], C, xt[:, 15 + t0 : 15 + t0 + PT],
                            start=False, stop=True,
                        )
                    for j, ps in enumerate(pss):
                        t0 = (g * GRP + j) * PT
                        if j % 2 == 0:
                            nc.vector.tensor_copy(
                                out=stage[:, t0 : t0 + PT], in_=ps[:]
                            )
                        else:
                            nc.scalar.copy(
                                out=stage[:, t0 : t0 + PT], in_=ps[:]
                            )

                nc.scalar.dma_start(
                    out=yT[:, YPAD + m * MT : YPAD + (m + 1) * MT],
                    in_=stage[:],
                )

    nc.compile()  # bacc legalization: <=1 sync wait per HW instruction
    return nc


def _get_nc():
    global _NC_CACHE
    if _NC_CACHE is None:
        _NC_CACHE = _build_nc()
    return _NC_CACHE


# ------------------------------------------------------------- host helpers
def _fir_mat(taps):
    """128x128 matrix of one masked FIR pass: y = T @ x (first 128 samples).

    y[i] = x[i] + sum_j taps[j] * x[i-j-1] for i >= F, else 0.
    """
    T = np.zeros((128, 128))
    for i in range(F, 128):
        T[i, i] = 1.0
        for j in range(F):
            T[i, i - j - 1] += taps[j]
    return T


def _build_weights(h64):
    """Stationary operands, stacked as [A C M0], fp16."""
    k1 = np.concatenate([[1.0], h64])
    k2 = np.concatenate([[1.0], h64[::-1]])
    kc = np.convolve(k1, k2)  # 33 taps

    i = np.arange(128)
    D = i[None, :] - i[:, None]  # po - p
    A = np.zeros((128, 128))
    mask = (D >= 0) & (D <= 32)
    A[mask] = kc[D[mask]]
    Dc = D + 128
    C = np.zeros((128, 128))
    maskc = (Dc >= 0) & (Dc <= 32)
    C[maskc] = kc[Dc[maskc]]

    M0 = (_fir_mat(h64[::-1]) @ _fir_mat(h64)).T  # M0[p, po] = dy[po]/dx[p]

    wm = np.zeros((128, 384), np.float16)
    wm[:, 0:128] = A
    wm[:, 128:256] = C
    wm[:, 256:384] = M0
    return wm


def _make_in_maps(x, h64):
    wm = _build_weights(h64)
    in_maps = []
    for c in range(N_CORES):
        xs = np.ascontiguousarray(x[c * RPC : (c + 1) * RPC])  # (32, 131072)
        Bv = xs.reshape(RPC, BPR, BLK)
        xin = np.zeros((128, 1 + DCOLS), np.float16)
        xin[:, 1:] = Bv.transpose(2, 0, 1).reshape(BLK, DCOLS)
        xpk = np.zeros((128, XCOLS), np.float16)
        for m in range(NMT):
            # halo col at slot 15, current cols at slots 16..16+MT-1
            xpk[:, m * W_IN + 15 : (m + 1) * W_IN] = (
                xin[:, m * MT : m * MT + MT + 1]
            )
        xpk[:, NMT * W_IN : XCOLS] = Bv[:, 0, :].astype(np.float16).T
        in_maps.append({"xp": xpk, "wm": wm})
    return in_maps


def _assemble(results):
    y = np.empty((B, N), np.float32)
    for c in range(N_CORES):
        yT = results[c]["yT"].astype(np.float32)
        main = (
            yT[:, YPAD : YPAD + DCOLS]
            .reshape(BLK, RPC, BPR)
            .transpose(1, 2, 0)
            .reshape(RPC, N)
        )
        y[c * RPC : (c + 1) * RPC] = main
        y[c * RPC : (c + 1) * RPC, 0:BLK] = yT[:, YPAD + DCOLS : YCOLS].T
    return y


def _run(x, h, **spmd_kwargs):
    x = np.asarray(x, dtype=np.float32)
    h64 = np.asarray(h, dtype=np.float64).reshape(-1)
    assert x.shape == (B, N) and h64.shape == (F,)
    in_maps = _make_in_maps(x, h64)
    res = run_bass_kernel_spmd(
        _get_nc(), in_maps, core_ids=list(range(N_CORES)), **spmd_kwargs
    )
    return _assemble(res.results), res


def kernel(x, kernel):
    y, _ = _run(x, kernel)
    return y
